# revision 15
# baseline (speedup 1.0000x reference)
"""MoD router kernel for Trainium2 (Bass/Tile), 8 NeuronCores, batch-parallel.

Problem (per batch b of 8):
    scores = x[b] @ w_router                       # (4096,)
    topk_scores, idx = top_k(scores, 3072)         # sorted desc
    routed = x[b][idx]                             # (3072, 1024)
    w = softmax(topk_scores)[:, None]
    blended = processed[b] * w + (1 - w) * routed
    out[b] = x[b];  out[b][idx] = blended

Key identity used here (no sort needed): position p with rank
r_p = #{j : s_j > s_p} is selected iff r_p < K, its blend weight is
exp(s_p - m) / Z with Z summed over selected positions, and it blends
with row `processed[r_p]`.  So we need ranks (O(N^2) counting on the
128-lane engines), an indirect row gather, and an elementwise blend.
"""

import numpy as np

import concourse.bacc as bacc
import concourse.bass as bass
import concourse.bass_isa as bass_isa
import concourse.mybir as mybir
from concourse.bass import IndirectOffsetOnAxis
from concourse.masks import make_identity
from concourse.tile import TileContext

B, S, D, K = 8, 4096, 1024, 3072
P = 128
G = S // P  # 32 position groups of 128
FP32 = mybir.dt.float32
BF16 = mybir.dt.bfloat16
I32 = mybir.dt.int32

# --- tunables -----------------------------------------------------------
LOAD_GPB = 4       # x-load groups per DMA (4 -> 2 MiB per dma_start)
STORE_GPB = 4      # out-store groups per DMA
RANK_JV = 2048     # rank compare columns handled by VectorE (rest: ScalarE)
SCORE_V_MOD = 1    # score groups: g % SCORE_V_MOD == 0 -> VectorE, else GpSimd
CHUNK = 8          # groups per rank-fixup chunk


def build_nc() -> bass.Bass:
    nc = bacc.Bacc("TRN2", target_bir_lowering=False, num_devices=B)

    x = nc.dram_tensor("x", [S, D], FP32, kind="ExternalInput").ap()
    proc = nc.dram_tensor("proc", [K, D], FP32, kind="ExternalInput").ap()
    w_in = nc.dram_tensor("w", [1, D], FP32, kind="ExternalInput").ap()
    out = nc.dram_tensor("out", [S, D], FP32, kind="ExternalOutput").ap()

    alu = mybir.AluOpType
    act = mybir.ActivationFunctionType

    with TileContext(nc) as tc:
        with (
            tc.tile_pool(name="persist", bufs=1) as pp,
            tc.tile_pool(name="scorescratch", bufs=2) as scp,
            tc.tile_pool(name="cmpv", bufs=2) as cvp,
            tc.tile_pool(name="cmpg", bufs=2) as cgp,
            tc.tile_pool(name="proctile", bufs=4) as prp,
            tc.tile_pool(name="psum_t", bufs=2, space="PSUM") as ptp,
            tc.tile_pool(name="psum_w", bufs=2, space="PSUM") as pwp,
        ):
            # ---- persistent tiles ----
            x_sb = pp.tile([P, G, D], FP32)        # 128 KiB/part
            sbc = pp.tile([P, S], FP32)            # scores bcast to all parts
            wbc = pp.tile([P, D], FP32)            # router weights bcast
            ident = pp.tile([P, P], FP32)
            ones = pp.tile([1, P], FP32)
            w_sb = pp.tile([1, D], FP32)
            s_col = pp.tile([P, G], FP32)          # s[g*128+p] at [p, g]
            rank_v = pp.tile([P, G], FP32)
            sgn_s = pp.tile([P, G], FP32)
            neg_s = pp.tile([P, G], FP32)
            cfix = pp.tile([P, G], FP32)
            rank = pp.tile([P, G], FP32)
            sel = pp.tile([P, G], FP32)
            e_col = pp.tile([P, G], FP32)
            em = pp.tile([P, G], FP32)
            w_col = pp.tile([P, G], FP32)
            omw = pp.tile([P, G], FP32)
            gidxf = pp.tile([P, G], FP32)
            gidx = pp.tile([P, G], I32)
            m_part = pp.tile([P, 1], FP32)
            m_all = pp.tile([P, 1], FP32)
            negm = pp.tile([P, 1], FP32)
            z_part = pp.tile([P, 1], FP32)
            z_all = pp.tile([P, 1], FP32)
            z_inv = pp.tile([P, 1], FP32)

            # ---- constants ----
            make_identity(nc, ident)
            nc.vector.memset(ones, 1.0)

            # router weights: DMA one row, broadcast to 128 partitions via PE
            nc.sync.dma_start(out=w_sb, in_=w_in)
            for h in range(2):
                pw = pwp.tile([P, D // 2], FP32, tag="pw")
                nc.tensor.matmul(
                    out=pw, lhsT=ones, rhs=w_sb[:, h * 512:(h + 1) * 512],
                    start=True, stop=True,
                )
                nc.scalar.copy(out=wbc[:, h * 512:(h + 1) * 512], in_=pw)

            # ---- phase 1: load x, compute scores ----
            for c in range(G // LOAD_GPB):
                g0 = c * LOAD_GPB
                src = x[g0 * P:(g0 + LOAD_GPB) * P, :].rearrange(
                    "(g p) d -> p g d", p=P
                )
                nc.sync.dma_start(out=x_sb[:, g0:g0 + LOAD_GPB, :], in_=src)

            for g in range(G):
                eng = nc.vector if g % SCORE_V_MOD == 0 else nc.gpsimd
                scr = scp.tile([P, D], FP32, tag="scr")
                eng.scalar_tensor_tensor(
                    out=scr, in0=x_sb[:, g, :], scalar=1.0, in1=wbc,
                    op0=alu.bypass, op1=alu.mult,
                    accum_out=s_col[:, g:g + 1],
                )

            # ---- phase 2a: broadcast scores to all partitions ----
            # transpose(s_col[:, g] broadcast along free) -> psum[j, p] = s[g*128+p]
            for c in range(G // 4):
                pst = ptp.tile([P, 4 * P], FP32, tag="pst")
                for k in range(4):
                    g = c * 4 + k
                    nc.tensor.transpose(
                        out=pst[:, k * P:(k + 1) * P],
                        in_=s_col[:, g:g + 1].to_broadcast([P, P]),
                        identity=ident,
                    )
                nc.scalar.copy(out=sbc[:, c * 4 * P:(c + 1) * 4 * P], in_=pst)

            # ---- phase 2b: ranks by counting, j-split across V and ScalarE ----
            # VectorE counts j < RANK_JV directly: sum of (s_j > s_i).
            # ScalarE covers j >= RANK_JV via Sign(s_j - s_i) accumulate:
            #   count_S = (sgn_sum + n_S - [i in S-half]) / 2   (no-tie case)
            n_s = S - RANK_JV
            g_split = RANK_JV // P  # first group whose positions sit in S-half
            nc.vector.tensor_scalar(
                out=neg_s, in0=s_col, scalar1=-1.0, scalar2=None, op0=alu.mult
            )
            nc.vector.memset(cfix[:, :g_split], n_s / 2.0)
            nc.vector.memset(cfix[:, g_split:], (n_s - 1) / 2.0)
            for g in range(G):
                cv = cvp.tile([P, RANK_JV], BF16, tag="cv")
                nc.vector.tensor_scalar(
                    out=cv, in0=sbc[:, :RANK_JV],
                    scalar1=s_col[:, g:g + 1], scalar2=None, op0=alu.is_gt,
                    op1=alu.add, accum_out=rank_v[:, g:g + 1],
                )
                cg = cgp.tile([P, n_s], BF16, tag="cg")
                nc.scalar.activation(
                    out=cg, in_=sbc[:, RANK_JV:], func=act.Sign,
                    bias=neg_s[:, g:g + 1],
                    accum_out=sgn_s[:, g:g + 1],
                )

            # softmax max (global max == max of top-k since top-k holds the max)
            nc.vector.tensor_reduce(
                out=m_part, in_=s_col, axis=mybir.AxisListType.X, op=alu.max
            )
            # cross-partition max via PE transpose of the broadcast column
            pm = ptp.tile([P, P], FP32, tag="pall")
            nc.tensor.transpose(
                out=pm, in_=m_part[:, 0:1].to_broadcast([P, P]), identity=ident
            )
            nc.vector.tensor_reduce(
                out=m_all, in_=pm, axis=mybir.AxisListType.X, op=alu.max
            )
            nc.vector.tensor_scalar(
                out=negm, in0=m_all, scalar1=-1.0, scalar2=None, op0=alu.mult
            )
            nc.scalar.activation(
                out=e_col, in_=s_col, func=act.Exp, bias=negm[:, 0:1]
            )

            for cc in range(G // CHUNK):
                cs = slice(cc * CHUNK, (cc + 1) * CHUNK)
                # rank = rank_v + 0.5*sgn_s + cfix
                nc.vector.scalar_tensor_tensor(
                    out=rank[:, cs], in0=sgn_s[:, cs], scalar=0.5,
                    in1=rank_v[:, cs], op0=alu.mult, op1=alu.add,
                )
                nc.vector.tensor_tensor(
                    out=rank[:, cs], in0=rank[:, cs], in1=cfix[:, cs],
                    op=alu.add,
                )
                nc.vector.tensor_scalar(
                    out=sel[:, cs], in0=rank[:, cs], scalar1=float(K),
                    scalar2=None, op0=alu.is_lt,
                )
                nc.vector.tensor_scalar(
                    out=gidxf[:, cs], in0=rank[:, cs], scalar1=float(K - 1),
                    scalar2=None, op0=alu.min,
                )
                nc.vector.tensor_copy(out=gidx[:, cs], in_=gidxf[:, cs])
                nc.vector.tensor_tensor(
                    out=em[:, cs], in0=e_col[:, cs], in1=sel[:, cs], op=alu.mult
                )

            # Z and weights
            nc.vector.tensor_reduce(
                out=z_part, in_=em, axis=mybir.AxisListType.X, op=alu.add
            )
            pz = ptp.tile([P, P], FP32, tag="pall")
            nc.tensor.transpose(
                out=pz, in_=z_part[:, 0:1].to_broadcast([P, P]), identity=ident
            )
            nc.vector.tensor_reduce(
                out=z_all, in_=pz, axis=mybir.AxisListType.X, op=alu.add
            )
            nc.vector.reciprocal(out=z_inv, in_=z_all)
            nc.vector.tensor_scalar(
                out=w_col, in0=em, scalar1=z_inv[:, 0:1], scalar2=None,
                op0=alu.mult,
            )
            nc.vector.tensor_scalar(
                out=omw, in0=w_col, scalar1=-1.0, scalar2=1.0,
                op0=alu.mult, op1=alu.add,
            )

            # ---- phase 3: gather processed[rank], blend, store ----
            for g in range(G):
                pt = prp.tile([P, D], FP32, tag="pt")
                nc.gpsimd.indirect_dma_start(
                    out=pt, out_offset=None, in_=proc,
                    in_offset=IndirectOffsetOnAxis(ap=gidx[:, g:g + 1], axis=0),
                )
                # pt <- w * proc   (ScalarE, per-partition scale)
                nc.scalar.activation(
                    out=pt, in_=pt, func=act.Copy, scale=w_col[:, g:g + 1]
                )
                # x_sb[g] <- (1-w) * x + pt   (in place; x_g dead afterwards)
                nc.vector.scalar_tensor_tensor(
                    out=x_sb[:, g, :], in0=x_sb[:, g, :],
                    scalar=omw[:, g:g + 1], in1=pt,
                    op0=alu.mult, op1=alu.add,
                )
                if (g + 1) % STORE_GPB == 0:
                    g0 = g + 1 - STORE_GPB
                    dst = out[g0 * P:(g + 1) * P, :].rearrange(
                        "(g p) d -> p g d", p=P
                    )
                    nc.sync.dma_start(out=dst, in_=x_sb[:, g0:g + 1, :])

    nc.compile()
    return nc


_NC_CACHE: bass.Bass | None = None


def _get_nc() -> bass.Bass:
    global _NC_CACHE
    if _NC_CACHE is None:
        _NC_CACHE = build_nc()
    return _NC_CACHE


def kernel(x: np.ndarray, processed: np.ndarray, w_router: np.ndarray,
           **run_kwargs) -> np.ndarray:
    from concourse.bass_utils import run_bass_kernel_spmd

    x = np.ascontiguousarray(x, dtype=np.float32)
    processed = np.ascontiguousarray(processed, dtype=np.float32)
    w2d = np.ascontiguousarray(w_router.reshape(1, D), dtype=np.float32)

    nc = _get_nc()
    in_maps = [
        {"x": x[b], "proc": processed[b], "w": w2d} for b in range(B)
    ]
    res = run_bass_kernel_spmd(nc, in_maps, core_ids=list(range(B)),
                               **run_kwargs)
    out = np.stack([res.results[b]["out"] for b in range(B)])
    kernel.last_results = res
    return out


# revision 37
# speedup vs baseline: 174.8286x; 174.8286x over previous
"""MoD router kernel for Trainium2 (Bass/Tile), 8 NeuronCores, batch-parallel.

Problem (per batch b of 8):
    scores = x[b] @ w_router                       # (4096,)
    topk_scores, idx = top_k(scores, 3072)         # sorted desc
    routed = x[b][idx]                             # (3072, 1024)
    w = softmax(topk_scores)[:, None]
    blended = processed[b] * w + (1 - w) * routed
    out[b] = x[b];  out[b][idx] = blended

Key identity used here (no sort needed): position p with rank
r_p = #{j : s_j > s_p} is selected iff r_p < K, its blend weight is
exp(s_p - m) / Z with Z summed over selected positions, and it blends
with row `processed[r_p]`.  So we need ranks (O(N^2) counting on the
128-lane engines), an indirect row gather, and an elementwise blend.

Engine split / schedule:
  - VectorE: scores (fused mul+accum) while x streams in, then rank
    counting over the HIGH half of columns (is_gt + accum, 2x mode),
    then the blend adds.
  - ScalarE: rank counting over the LOW half via Sign(s_j - s_i)
    accumulate — the low columns and their neg-score biases are
    produced first, so ScalarE's counting starts while x is still
    loading; later the blend scales.
  - PE: per-group transpose broadcast of scores, w_router broadcast,
    cross-partition reductions.
  - GpSimd/SWDGE: indirect row gathers of processed[rank], issued per
    fixup chunk so they overlap the rank phase.
"""

import numpy as np

import concourse.bacc as bacc
import concourse.bass as bass
import concourse.mybir as mybir
from concourse.bass import IndirectOffsetOnAxis
from concourse.masks import make_identity
from concourse.tile import TileContext

B, S, D, K = 8, 4096, 1024, 3072
P = 128
G = S // P           # 32 position groups of 128
FP32 = mybir.dt.float32
BF16 = mybir.dt.bfloat16
I32 = mybir.dt.int32

# --- tunables -----------------------------------------------------------
LOAD_CHUNKS = [2, 2, 4, 4, 4, 4, 4, 4, 4]  # x-load groups per DMA
NS = 1920            # rank columns on ScalarE (low half); VectorE gets S-NS
G_SPLIT = NS // P    # groups whose positions fall in the ScalarE half
CHUNK = 4            # groups per rank-fixup / gather chunk
PT_BUFS = 14         # gather tile buffers (bf16)


def build_nc() -> bass.Bass:
    nc = bacc.Bacc("TRN2", target_bir_lowering=False, num_devices=B)

    x = nc.dram_tensor("x", [S, D], FP32, kind="ExternalInput").ap()
    proc = nc.dram_tensor("proc", [K, D], FP32, kind="ExternalInput").ap()
    w_in = nc.dram_tensor("w", [1, D], FP32, kind="ExternalInput").ap()
    out = nc.dram_tensor("out", [S, D], FP32, kind="ExternalOutput").ap()

    alu = mybir.AluOpType
    act = mybir.ActivationFunctionType
    NV = S - NS  # vector-side rank columns
    pt_tiles = {}

    with TileContext(nc) as tc:
        with (
            tc.tile_pool(name="persist", bufs=1) as pp,
            tc.tile_pool(name="scorescratch", bufs=1) as scp,
            tc.tile_pool(name="cmpv", bufs=1) as cvp,
            tc.tile_pool(name="cmpg", bufs=1) as cgp,
            tc.tile_pool(name="proctile", bufs=PT_BUFS) as prp,
            tc.tile_pool(name="outtile", bufs=4) as otp,
            tc.tile_pool(name="psum_t", bufs=2, space="PSUM") as ptp,
            tc.tile_pool(name="psum_w", bufs=2, space="PSUM") as pwp,
        ):
            # ---- persistent tiles ----
            x_sb = pp.tile([P, G, D], FP32)        # 128 KiB/part
            sbc_lo = pp.tile([P, NS], FP32)        # score bcast, cols [0, NS)
            sbc_hi = pp.tile([P, NV], FP32)        # score bcast, cols [NS, S)
            wbc = pp.tile([P, D], FP32)            # router weights bcast
            ident = pp.tile([P, P], FP32)
            ones = pp.tile([1, P], FP32)
            # w_sb is dead once wbc is built; share the score-scratch slot
            w_sb = scp.tile([1, D], FP32, tag="scr")
            s_col = pp.tile([P, G], FP32)          # s[g*128+p] at [p, g]
            neg_s = pp.tile([P, G], FP32)
            rank_v = pp.tile([P, G], FP32)
            sgn_s = pp.tile([P, G], FP32)
            cfix = pp.tile([P, G], FP32)
            rank = pp.tile([P, G], FP32)
            e_col = pp.tile([P, G], FP32)
            em = pp.tile([P, G], FP32)
            w_col = pp.tile([P, G], FP32)
            omw = pp.tile([P, G], FP32)
            gidx = pp.tile([P, G], I32)
            m_part = pp.tile([P, 1], FP32)
            m_all = pp.tile([P, 1], FP32)
            negm = pp.tile([P, 1], FP32)
            z_part = pp.tile([P, 1], FP32)
            z_all = pp.tile([P, 1], FP32)
            z_inv = pp.tile([P, 1], FP32)

            # ---- constants ----
            make_identity(nc, ident)
            nc.vector.memset(ones, 1.0)
            nc.vector.memset(cfix[:, :G_SPLIT], (NS - 1) / 2.0)
            nc.vector.memset(cfix[:, G_SPLIT:], NS / 2.0)

            # router weights: DMA one row, broadcast to 128 partitions via PE
            nc.sync.dma_start(out=w_sb, in_=w_in)
            for h in range(2):
                pw = pwp.tile([P, D // 2], FP32, tag="pw")
                nc.tensor.matmul(
                    out=pw, lhsT=ones, rhs=w_sb[:, h * 512:(h + 1) * 512],
                    start=True, stop=True,
                )
                nc.scalar.copy(out=wbc[:, h * 512:(h + 1) * 512], in_=pw)

            # ---- x loads (HWDGE; first chunks smaller so scores start early)
            g0 = 0
            for n in LOAD_CHUNKS:
                src = x[g0 * P:(g0 + n) * P, :].rearrange(
                    "(g p) d -> p g d", p=P
                )
                nc.sync.dma_start(out=x_sb[:, g0:g0 + n, :], in_=src)
                g0 += n

            # ---- scores + score broadcast, in chunks of 4 groups ----
            def score_chunk(c):
                for k in range(4):
                    g = c * 4 + k
                    scr = scp.tile([P, D], FP32, tag="scr")
                    nc.vector.scalar_tensor_tensor(
                        out=scr, in0=x_sb[:, g, :], scalar=1.0, in1=wbc,
                        op0=alu.bypass, op1=alu.mult,
                        accum_out=s_col[:, g:g + 1],
                    )
                pst = ptp.tile([P, 4 * P], FP32, tag="pst")
                for k in range(4):
                    g = c * 4 + k
                    nc.tensor.transpose(
                        out=pst[:, k * P:(k + 1) * P],
                        in_=s_col[:, g:g + 1].to_broadcast([P, P]),
                        identity=ident,
                    )
                col0 = c * 4 * P
                col1 = col0 + 4 * P
                if col0 < NS:
                    # low part feeds ScalarE's Sign counting — ACT copies it
                    # (emitted before any Sign op, so it wins priority)
                    e = min(col1, NS)
                    nc.scalar.copy(
                        out=sbc_lo[:, col0:e], in_=pst[:, :e - col0]
                    )
                if col1 > NS:
                    # high part feeds VectorE's counting; keep it off ACT so
                    # queued Sign ops can't starve it
                    s0 = max(col0, NS)
                    nc.vector.tensor_copy(
                        out=sbc_hi[:, s0 - NS:col1 - NS],
                        in_=pst[:, s0 - col0:],
                    )
                nc.vector.tensor_scalar(
                    out=neg_s[:, c * 4:(c + 1) * 4],
                    in0=s_col[:, c * 4:(c + 1) * 4],
                    scalar1=-1.0, scalar2=None, op0=alu.mult,
                )

            def sign_chunk(cc):
                # ScalarE count over the low columns:
                # count_S = (sum Sign(s_j - s_i) + NS - [i in lo]) / 2
                for k in range(CHUNK):
                    g = cc * CHUNK + k
                    cg = cgp.tile([P, NS], BF16, tag="cg")
                    nc.scalar.activation(
                        out=cg, in_=sbc_lo, func=act.Sign,
                        bias=neg_s[:, g:g + 1],
                        accum_out=sgn_s[:, g:g + 1],
                    )

            # score chunks needed before sbc_lo is complete
            lo_chunks = -(-NS // (4 * P))
            for c in range(lo_chunks):
                score_chunk(c)
            # sbc_lo complete -> ScalarE can start counting the low half
            # for the already-scored groups while x is still loading.
            for cc in range(lo_chunks * 4 // CHUNK):
                sign_chunk(cc)
            for c in range(lo_chunks, G // 4):
                score_chunk(c)
                # neg_s for these groups is now emitted; their Sign ops can go
                for cc in range(c * 4 // CHUNK, (c + 1) * 4 // CHUNK):
                    sign_chunk(cc)

            # softmax max (global max == max of top-k since top-k holds it)
            nc.vector.tensor_reduce(
                out=m_part, in_=s_col, axis=mybir.AxisListType.X, op=alu.max
            )
            pm = ptp.tile([P, P], FP32, tag="pall")
            nc.tensor.transpose(
                out=pm, in_=m_part[:, 0:1].to_broadcast([P, P]), identity=ident
            )
            nc.vector.tensor_reduce(
                out=m_all, in_=pm, axis=mybir.AxisListType.X, op=alu.max
            )
            nc.vector.tensor_scalar(
                out=negm, in0=m_all, scalar1=-1.0, scalar2=None, op0=alu.mult
            )
            nc.scalar.activation(
                out=e_col, in_=s_col, func=act.Exp, bias=negm[:, 0:1]
            )

            # ---- rank counting (VectorE, high half) + fixup + gathers ----
            for cc in range(G // CHUNK):
                for k in range(CHUNK):
                    g = cc * CHUNK + k
                    cv = cvp.tile([P, NV], BF16, tag="cv")
                    nc.vector.tensor_scalar(
                        out=cv, in0=sbc_hi,
                        scalar1=s_col[:, g:g + 1], scalar2=None, op0=alu.is_gt,
                        op1=alu.add, accum_out=rank_v[:, g:g + 1],
                    )
                cs = slice(cc * CHUNK, (cc + 1) * CHUNK)
                # rank = rank_v + 0.5*sgn + cfix
                nc.vector.scalar_tensor_tensor(
                    out=rank[:, cs], in0=sgn_s[:, cs], scalar=0.5,
                    in1=rank_v[:, cs], op0=alu.mult, op1=alu.add,
                )
                nc.vector.tensor_tensor(
                    out=rank[:, cs], in0=rank[:, cs], in1=cfix[:, cs],
                    op=alu.add,
                )
                nc.vector.tensor_scalar(
                    out=gidx[:, cs], in0=rank[:, cs], scalar1=float(K - 1),
                    scalar2=None, op0=alu.min,
                )
                # em = (rank < K) * e   in one fused op
                nc.vector.scalar_tensor_tensor(
                    out=em[:, cs], in0=rank[:, cs], scalar=float(K),
                    in1=e_col[:, cs], op0=alu.is_lt, op1=alu.mult,
                )
                # start this chunk's gathers immediately (need only gidx)
                for k in range(CHUNK):
                    g = cc * CHUNK + k
                    pt = prp.tile([P, D], BF16, tag="pt")
                    nc.gpsimd.indirect_dma_start(
                        out=pt, out_offset=None, in_=proc,
                        in_offset=IndirectOffsetOnAxis(
                            ap=gidx[:, g:g + 1], axis=0
                        ),
                    )
                    pt_tiles[g] = pt

            # Z and weights (needs all chunks)
            nc.vector.tensor_reduce(
                out=z_part, in_=em, axis=mybir.AxisListType.X, op=alu.add
            )
            pz = ptp.tile([P, P], FP32, tag="pall")
            nc.tensor.transpose(
                out=pz, in_=z_part[:, 0:1].to_broadcast([P, P]), identity=ident
            )
            nc.vector.tensor_reduce(
                out=z_all, in_=pz, axis=mybir.AxisListType.X, op=alu.add
            )
            nc.vector.reciprocal(out=z_inv, in_=z_all)
            nc.vector.tensor_scalar(
                out=w_col, in0=em, scalar1=z_inv[:, 0:1], scalar2=None,
                op0=alu.mult,
            )
            nc.vector.tensor_scalar(
                out=omw, in0=w_col, scalar1=-1.0, scalar2=1.0,
                op0=alu.mult, op1=alu.add,
            )

            # ---- blend + store ----
            for g in range(G):
                pt = pt_tiles[g]
                # pt <- w * proc   (ScalarE, per-partition scale)
                nc.scalar.activation(
                    out=pt, in_=pt, func=act.Copy, scale=w_col[:, g:g + 1]
                )
                ot = otp.tile([P, D], FP32, tag="ot")
                # ot = (1-w) * x + pt
                nc.vector.scalar_tensor_tensor(
                    out=ot, in0=x_sb[:, g, :],
                    scalar=omw[:, g:g + 1], in1=pt,
                    op0=alu.mult, op1=alu.add,
                )
                nc.sync.dma_start(out=out[g * P:(g + 1) * P, :], in_=ot)

    nc.compile()
    return nc


_NC_CACHE: bass.Bass | None = None


def _get_nc() -> bass.Bass:
    global _NC_CACHE
    if _NC_CACHE is None:
        _NC_CACHE = build_nc()
    return _NC_CACHE


def kernel(x: np.ndarray, processed: np.ndarray, w_router: np.ndarray,
           **run_kwargs) -> np.ndarray:
    from concourse.bass_utils import run_bass_kernel_spmd

    x = np.ascontiguousarray(x, dtype=np.float32)
    processed = np.ascontiguousarray(processed, dtype=np.float32)
    w2d = np.ascontiguousarray(w_router.reshape(1, D), dtype=np.float32)

    nc = _get_nc()
    in_maps = [
        {"x": x[b], "proc": processed[b], "w": w2d} for b in range(B)
    ]
    res = run_bass_kernel_spmd(nc, in_maps, core_ids=list(range(B)),
                               **run_kwargs)
    out = np.stack([res.results[b]["out"] for b in range(B)])
    kernel.last_results = res
    return out
